# revision 9
# baseline (speedup 1.0000x reference)
"""Batched 2048-point DFT on 8 TRN2 NeuronCores — radix-64/32 real-block.

n = 2048 = 64 * 32, m = 32*m1 + m2 (m1 in 64, m2 in 32), k = k1 + 64*k2.
  Z[b; k1, m2] = sum_m1 A_m2[m1,k1] * x[b, 32*m1+m2],
      A_m2[m1,k1] = exp(-2i*pi*(32*m1+m2)*k1/2048)
  X[b; k1+64*k2] = sum_m2 W32[m2,k2] * Z[b; k1, m2]

Stage 1 packs the complex arithmetic into the stationary: contraction runs
over (re|im, m1) = 128 partitions, the 128 stationary columns are
(re|im, k1), so every Z element is produced by ONE matmul row (16384 rows
per core vs 32768 for the 4-matmul complex form).  Stage 2 contracts m2
(on partitions after a PE transpose) with block-diag S = perm(I4 (x) W32):
  X = S_re.T @ T   (full 512-col matmul, start of PSUM group)
  X[re-cols] += (-S_im).T @ T[im-cols]   (accumulating half-matmuls with
  X[im-cols] += S_im.T @ T[re-cols]       the sign baked into a 3rd const)
so the complex combine happens inside PSUM and the evacuation is a plain
contiguous fp32->fp16 copy that either DVE or ACT can run.

Operands fp16, PSUM fp32 (TRN2 matmuls cannot write 16-bit PSUM).  All
input DMAs are issued on the otherwise-idle SP queue in strict FIFO order
(ident, A, S, then x octets) so output dumps never steal input bandwidth;
dma_start issue costs ~600ns each on every engine, so inputs use few
large transfers.  The batch is processed in two halves: phase 2 of half
0 overlaps the x DMA of half 1, with half-1 stage-1 pairs woven between
phase-2 groups (all emitted before half-1's first transpose, which reads
every m2 column of Z).  Transposes are emitted two groups ahead so the
PE never waits on the T8 evacuation chain; junk identity matmuls warm
the PE HAM throttle during the input fill (more junk measured slower --
power budget -- so the fill is deliberately not fully covered).

Per core (batch shard 512, b = h*256 + 2*b2 + b0):
  x SBUF  [128=(riI,m1), (h, m2, b'')]
  Z SBUF  [128=(riO,k1), (h, b2, m2, b0)]   (pair-interleaved b for packed
                                             2-byte strided evac writes)
  T       [128=(b2l,m2,b0), (chunk, riO, k1)] from PE transposes
  X SBUF  [128=(b2l,k2,b0), (chl, riX, k1)] -> contiguous DMA dump,
  host-side unscramble (free).
"""

import sys

for _p in ("/opt/trn_rl_repo", "/root/.axon_site/_ro/trn_rl_repo"):
    if _p not in sys.path:
        sys.path.insert(0, _p)

import numpy as np

import concourse.bass as bass
import concourse.mybir as mybir
import concourse.tile as tile
from concourse import bacc
from concourse.bass_utils import run_bass_kernel_spmd

BATCH = 4096
NFFT = 2048
NCORES = 8
BPC = BATCH // NCORES  # 512
N1 = 64  # stage-1 radix (contraction m1)
N2 = 32  # stage-2 radix (contraction m2)
NH = 2  # batch halves per core
BH = BPC // NH  # 256
NCH = BH // 4  # 64 transpose chunks per half
NGRP = NCH // 8  # 8 transpose/stage-2 groups per half
NWARM = 12  # warmup tiles (4 matmuls each) to lift the PE HAM throttle

F16 = mybir.dt.float16
F32 = mybir.dt.float32

_CACHE = {}


def _build_nc():
    nc = bacc.Bacc("TRN2", target_bir_lowering=False, debug=False)

    # constants: ident+smat merged in one small leading DMA (fewer issue
    # slots ahead of the x stream), then A
    c_d = nc.dram_tensor("cmat", [128, 4 * 128], F16, kind="ExternalInput").ap()
    a_d = nc.dram_tensor("amat", [128, N2 * 128], F16, kind="ExternalInput").ap()
    # x: [128=(riI,m1), (h, m2, b'')]
    x_d = nc.dram_tensor("xin", [128, NH * N2 * BH], F16, kind="ExternalInput").ap()
    # output dump: per group g: [128=(b2l,k2,b0), (chl 16, riX 2, k1 64)]
    o_d = nc.dram_tensor("odump", [NH * 8 * 128, 1024], F16, kind="ExternalOutput").ap()

    x_v = x_d.rearrange("p (h m b) -> p h m b", h=NH, m=N2)
    o_v = o_d.rearrange("(g p) c -> g p c", g=NH * 8)

    with tile.TileContext(nc) as tc:
        with (
            tc.tile_pool(name="const", bufs=1) as cpool,
            tc.tile_pool(name="x", bufs=1) as xpool,
            tc.tile_pool(name="z", bufs=1) as zpool,
            tc.tile_pool(name="t", bufs=4) as tpool,
            tc.tile_pool(name="o", bufs=3) as opool,
        ):
            c_t = cpool.tile([128, 4, 128], F16, tag="cmat")
            a_t = cpool.tile([128, N2, 128], F16, tag="amat")
            x_t = xpool.tile([128, NH, N2, BH], F16, tag="x")
            ident = c_t[:, 0, :]

            # One FIFO DMA queue on the otherwise-idle SP engine: consts
            # first, then x half 0, x half 1, and (later) the output dumps.
            # FIFO order keeps output transfers from stealing bandwidth
            # from the input stream.
            nc.sync.dma_start(c_t[:], c_d.rearrange("p (s c) -> p s c", s=4))
            nc.sync.dma_start(a_t[:], a_d.rearrange("p (m c) -> p m c", m=N2))
            for h in range(NH):
                for q in range(N2 // 8):
                    nc.sync.dma_start(
                        x_t[:, h, 8 * q : 8 * q + 8, :], x_v[:, h, 8 * q : 8 * q + 8, :]
                    )

            # Z [128=(riO,k1), (h, b2, m2, b0)]
            z_t = zpool.tile([128, NH * BH * N2], F16, tag="z")
            # view for pair-interleaved stage-1 evac writes (m2, b2, b0 order)
            z_w = z_t[:].rearrange(
                "p (h b2 m b0) -> p h m b2 b0", h=NH, b2=BH // 2, m=N2
            )
            # view for transpose chunk reads (contiguous 128-col chunks)
            z_r = z_t[:].rearrange("p (h c w) -> p h c w", h=NH, c=NCH)

            sgre = c_t[:, 1, :]
            sgim = c_t[:, 2, :]
            sgimn = c_t[:, 3, :]

            with (
                tc.tile_pool(name="pst", bufs=3, space="PSUM") as pstpool,
                tc.tile_pool(name="ps2", bufs=5, space="PSUM") as ps2pool,
            ):
                # ---- warmup: junk matmuls on the identity during DMA fill ----
                for _ in range(NWARM):
                    psw = ps2pool.tile([128, 2, BH], F32, tag="pq")
                    for rep in range(4):
                        nc.tensor.matmul(
                            psw[:, rep // 2, 128 * (rep % 2) : 128 * (rep % 2 + 1)],
                            ident,
                            ident,
                            start=True,
                            stop=True,
                        )

                def emit_warm(n):
                    for _ in range(n):
                        pw = ps2pool.tile([128, 2, BH], F32, tag="pq", name="pw")
                        for rep in range(4):
                            nc.tensor.matmul(
                                pw[:, rep // 2, 128 * (rep % 2) : 128 * (rep % 2 + 1)],
                                ident,
                                ident,
                                start=True,
                                stop=True,
                            )

                def emit_s1(h, j0=0, j1=N2 // 2, weave_warm=False):
                    # stage 1 for half h: per m2-pair one PSUM tile + one evac
                    for j in range(j0, j1):
                        if weave_warm and j in (3, 6):
                            emit_warm(1)
                        ps = ps2pool.tile([128, 2, BH], F32, tag="pq")
                        nc.tensor.matmul(
                            ps[:, 0, :], a_t[:, 2 * j, :], x_t[:, h, 2 * j, :],
                            start=True, stop=True,
                        )
                        nc.tensor.matmul(
                            ps[:, 1, :], a_t[:, 2 * j + 1, :], x_t[:, h, 2 * j + 1, :],
                            start=True, stop=True,
                        )
                        # evac: in (m2l, b2, b0) contiguous; out pair-strided
                        src = ps[:].rearrange("p m (b2 b0) -> p m b2 b0", b0=2)
                        dst = z_w[:, h, 2 * j : 2 * j + 2, :, :]
                        if j % 2 == 0:
                            nc.vector.tensor_copy(dst, src)
                        else:
                            nc.scalar.copy(dst, src)

                def emit_tp(h, t):
                    # 8 transposes into one PSUM bank + one evac to T8.
                    # Transposes depend only on Z, so emitting them a group
                    # ahead keeps the PE fed while stage-2 waits on T8 evacs.
                    pt = pstpool.tile([128, 8, 128], F16, tag="pt")
                    for jj in range(8):
                        cl = t * 8 + jj
                        nc.tensor.transpose(
                            pt[:, jj, :], z_r[:, h, cl, :], ident
                        )
                    t8 = tpool.tile([128, 8 * 128], F16, tag="t8")
                    nc.vector.tensor_copy(t8[:], pt[:])
                    return t8

                def emit_s2(h, t, t8):
                    # 6 stage-2 matmuls (same-stationary batched to halve
                    # weight reloads), 2 plain evacs, one out-DMA
                    t8v = t8[:].rearrange("p (jj ri k) -> p jj ri k", jj=8, ri=2)
                    x_o = opool.tile([128, 8, 2, 64], F16, tag="xo")
                    pss = [
                        ps2pool.tile([128, 4, 2, 64], F32, tag="pq", name=f"pq_{h}_{t}_{s}")
                        for s in range(2)
                    ]
                    # X = S_re.T @ T, then accumulate the cross terms:
                    #   re-cols += (-S_im).T @ T[im-cols]
                    #   im-cols += S_im.T @ T[re-cols]
                    for s in range(2):
                        nc.tensor.matmul(
                            pss[s][:], sgre, t8[:, 512 * s : 512 * s + 512],
                            start=True, stop=False, skip_group_check=True,
                        )
                    for s in range(2):
                        nc.tensor.matmul(
                            pss[s][:, :, 0, :], sgimn,
                            t8v[:, 4 * s : 4 * s + 4, 1, :],
                            start=False, stop=False, skip_group_check=True,
                        )
                    for s in range(2):
                        nc.tensor.matmul(
                            pss[s][:, :, 1, :], sgim,
                            t8v[:, 4 * s : 4 * s + 4, 0, :],
                            start=False, stop=True, skip_group_check=True,
                        )
                    g = h * 8 + t
                    xof = x_o[:].rearrange("p c ri k -> p (c ri k)")
                    if (h, t) == (1, 7):
                        # final group: quarter evacs on DVE+ACT in parallel
                        # and 64KB dumps so the last-DMA-complete (the
                        # measured end of the kernel) lands ASAP
                        nc.vector.tensor_copy(x_o[:, 0:2, :, :], pss[0][:, 0:2])
                        nc.scalar.copy(x_o[:, 2:4, :, :], pss[0][:, 2:4])
                        nc.sync.dma_start(o_v[g][:, 0:256], xof[:, 0:256])
                        nc.gpsimd.dma_start(o_v[g][:, 256:512], xof[:, 256:512])
                        nc.vector.tensor_copy(x_o[:, 4:6, :, :], pss[1][:, 0:2])
                        nc.scalar.copy(x_o[:, 6:8, :, :], pss[1][:, 2:4])
                        nc.sync.dma_start(o_v[g][:, 512:768], xof[:, 512:768])
                        nc.gpsimd.dma_start(o_v[g][:, 768:1024], xof[:, 768:1024])
                    else:
                        if t % 2 == 0:
                            nc.vector.tensor_copy(x_o[:, 0:4, :, :], pss[0][:])
                        else:
                            nc.scalar.copy(x_o[:, 0:4, :, :], pss[0][:])
                        nc.sync.dma_start(o_v[g][:, 0:512], xof[:, 0:512])
                        nc.scalar.copy(x_o[:, 4:8, :, :], pss[1][:])
                        nc.gpsimd.dma_start(o_v[g][:, 512:1024], xof[:, 512:1024])

                # half 0 stage 1, then phase-2 h0 groups 0..3, then stage-1 h1
                # (x-h1 DMA has landed by then), then the rest
                emit_s1(0, weave_warm=True)
                sched = [(0, t) for t in range(8)] + [(1, t) for t in range(8)]
                from collections import deque

                t8q = deque()
                t8q.append(emit_tp(0, 0))
                t8q.append(emit_tp(0, 1))
                # all 16 h1 m2-pairs must be emitted before emit_tp(1, 0)
                # (at i=6) -- transposes read every m2 column of Z
                s1h1_plan = {0: (0, 2), 1: (2, 4), 2: (4, 7), 3: (7, 10), 4: (10, 13), 5: (13, 16)}
                for i, (h, t) in enumerate(sched):
                    if h == 0 and t in s1h1_plan:
                        emit_s1(1, *s1h1_plan[t])
                    if i + 2 < len(sched):
                        t8q.append(emit_tp(*sched[i + 2]))
                    emit_s2(h, t, t8q.popleft())

    nc.compile()
    return nc


def _consts():
    m1 = np.arange(N1, dtype=np.float64)
    k1 = np.arange(N1, dtype=np.float64)
    m2 = np.arange(N2, dtype=np.float64)
    k2 = np.arange(N2, dtype=np.float64)

    # amat[p=(riI*64+m1), m2*128 + (riO*64+k1)]
    # A_m2 = exp(-i*th), th = 2pi(32*m1+m2)k1/2048: Are=cos th, Aim=-sin th
    amat = np.empty((128, N2, 128), np.float64)
    for q in range(N2):
        th = 2.0 * np.pi * np.outer(32.0 * m1 + q, k1) / NFFT
        are = np.cos(th)
        aim = -np.sin(th)
        amat[0:64, q, 0:64] = are
        amat[64:128, q, 0:64] = -aim
        amat[0:64, q, 64:128] = aim
        amat[64:128, q, 64:128] = are

    # smat[w=(b2l*64+m2*2+b0), s*128 + (b2l'*64+k2*2+b0')]
    # W32 = exp(-i*phi), phi = 2pi*m2*k2/32: Wre=cos, Wim=-sin
    phi = 2.0 * np.pi * np.outer(m2, k2) / N2
    wre = np.cos(phi)
    wim = -np.sin(phi)
    w = np.arange(128)
    b2l_r = w // 64
    m2_r = (w % 64) // 2
    b0_r = w % 2
    cx = np.arange(128)
    b2l_c = cx // 64
    k2_c = (cx % 64) // 2
    b0_c = cx % 2
    mask = (b2l_r[:, None] == b2l_c[None, :]) & (b0_r[:, None] == b0_c[None, :])
    smat = np.zeros((128, 3, 128), np.float64)
    smat[:, 0, :] = wre[np.ix_(m2_r, k2_c)] * mask
    smat[:, 1, :] = wim[np.ix_(m2_r, k2_c)] * mask
    smat[:, 2, :] = -smat[:, 1, :]

    ident = np.eye(128, dtype=np.float64)
    cmat = np.concatenate([ident[:, :, None].transpose(0, 2, 1), smat], axis=1)
    return (
        np.ascontiguousarray(cmat.reshape(128, 4 * 128)).astype(np.float16),
        np.ascontiguousarray(amat.reshape(128, N2 * 128)).astype(np.float16),
    )


def run(signal_re, signal_im, trace=False, tmpdir=None):
    if "nc" not in _CACHE:
        _CACHE["nc"] = _build_nc()
        _CACHE["c"] = _consts()
    nc = _CACHE["nc"]
    cmat, amat = _CACHE["c"]

    sre = np.asarray(signal_re, dtype=np.float32).astype(np.float16)
    sim = np.asarray(signal_im, dtype=np.float32).astype(np.float16)

    in_maps = []
    for c in range(NCORES):
        bsl = slice(c * BPC, (c + 1) * BPC)
        # xin[riI*64+m1, h*8192 + m2*256 + b''] = x_ri[h*256+b'', 32*m1+m2]
        xr = sre[bsl].reshape(NH, BH, N1, N2)  # [h, b'', m1, m2]
        xi = sim[bsl].reshape(NH, BH, N1, N2)
        x = np.stack([xr, xi], axis=0)  # [ri, h, b'', m1, m2]
        x = x.transpose(0, 3, 1, 4, 2)  # [ri, m1, h, m2, b'']
        xin = np.ascontiguousarray(x.reshape(128, NH * N2 * BH))
        in_maps.append({"cmat": cmat, "amat": amat, "xin": xin})

    last_exc = None
    for attempt in range(3):
        try:
            br = run_bass_kernel_spmd(
                nc, in_maps, list(range(NCORES)), trace=trace, tmpdir=tmpdir
            )
            break
        except Exception as e:
            last_exc = e
            import time

            time.sleep(2.0)
    else:
        raise last_exc

    out_re = np.empty((BATCH, NFFT), np.float32)
    out_im = np.empty((BATCH, NFFT), np.float32)
    for c in range(NCORES):
        bsl = slice(c * BPC, (c + 1) * BPC)
        # odump[g*128 + (b2l*64+k2*2+b0), chl*128 + riX*64 + k1]
        # b = 4*(g*8+chl) + 2*b2l + b0 ; k = k1 + 64*k2
        d = br.results[c]["odump"].reshape(16, 2, 32, 2, 8, 2, 64)
        # dims: [g, b2l, k2, b0, chl, riX, k1]
        arr = d.transpose(5, 0, 4, 1, 3, 2, 6).reshape(2, BPC, NFFT)
        out_re[bsl, :] = arr[0].astype(np.float32)
        out_im[bsl, :] = arr[1].astype(np.float32)
    return (out_re, out_im), br


def kernel(signal_re, signal_im):
    return run(signal_re, signal_im)[0]


# revision 10
# speedup vs baseline: 1.0131x; 1.0131x over previous
"""Batched 2048-point DFT on 8 TRN2 NeuronCores — radix-64/32 real-block.

n = 2048 = 64 * 32, m = 32*m1 + m2 (m1 in 64, m2 in 32), k = k1 + 64*k2.
  Z[b; k1, m2] = sum_m1 A_m2[m1,k1] * x[b, 32*m1+m2],
      A_m2[m1,k1] = exp(-2i*pi*(32*m1+m2)*k1/2048)
  X[b; k1+64*k2] = sum_m2 W32[m2,k2] * Z[b; k1, m2]

Stage 1 packs the complex arithmetic into the stationary: contraction runs
over (re|im, m1) = 128 partitions, the 128 stationary columns are
(re|im, k1), so every Z element is produced by ONE matmul row (16384 rows
per core vs 32768 for the 4-matmul complex form).  Stage 2 contracts m2
(on partitions after a PE transpose) with block-diag S = perm(I4 (x) W32):
  X = S_re.T @ T   (full 512-col matmul, start of PSUM group)
  X[re-cols] += (-S_im).T @ T[im-cols]   (accumulating half-matmuls with
  X[im-cols] += S_im.T @ T[re-cols]       the sign baked into a 3rd const)
so the complex combine happens inside PSUM and the evacuation is a plain
contiguous fp32->fp16 copy that either DVE or ACT can run.

Operands fp16, PSUM fp32 (TRN2 matmuls cannot write 16-bit PSUM).  All
input DMAs are issued on the otherwise-idle SP queue in strict FIFO order
(ident+S merged in one leading 128KB DMA, then A, then x octets) so
output dumps never steal input bandwidth;
dma_start issue costs ~600ns each on every engine, so inputs use few
large transfers.  The batch is processed in two halves: phase 2 of half
0 overlaps the x DMA of half 1, with half-1 stage-1 pairs woven between
phase-2 groups (all emitted before half-1's first transpose, which reads
every m2 column of Z).  Transposes are emitted two groups ahead so the
PE never waits on the T8 evacuation chain; junk identity matmuls warm
the PE HAM throttle during the input fill (more junk measured slower --
power budget -- so the fill is deliberately not fully covered).

Per core (batch shard 512, b = h*256 + 2*b2 + b0):
  x SBUF  [128=(riI,m1), (h, m2, b'')]
  Z SBUF  [128=(riO,k1), (h, b2, m2, b0)]   (pair-interleaved b for packed
                                             2-byte strided evac writes)
  T       [128=(b2l,m2,b0), (chunk, riO, k1)] from PE transposes
  X SBUF  [128=(b2l,k2,b0), (chl, riX, k1)] -> contiguous DMA dump,
  host-side unscramble (free).
"""

import sys

for _p in ("/opt/trn_rl_repo", "/root/.axon_site/_ro/trn_rl_repo"):
    if _p not in sys.path:
        sys.path.insert(0, _p)

import numpy as np

import concourse.bass as bass
import concourse.mybir as mybir
import concourse.tile as tile
from concourse import bacc
from concourse.bass_utils import run_bass_kernel_spmd

BATCH = 4096
NFFT = 2048
NCORES = 8
BPC = BATCH // NCORES  # 512
N1 = 64  # stage-1 radix (contraction m1)
N2 = 32  # stage-2 radix (contraction m2)
NH = 2  # batch halves per core
BH = BPC // NH  # 256
NCH = BH // 4  # 64 transpose chunks per half
NGRP = NCH // 8  # 8 transpose/stage-2 groups per half
NWARM = 12  # warmup tiles (4 matmuls each) to lift the PE HAM throttle

F16 = mybir.dt.float16
F32 = mybir.dt.float32

_CACHE = {}


def _build_nc():
    nc = bacc.Bacc("TRN2", target_bir_lowering=False, debug=False)

    # constants: ident+smat merged in one small leading DMA (fewer issue
    # slots ahead of the x stream), then A
    c_d = nc.dram_tensor("cmat", [128, 4 * 128], F16, kind="ExternalInput").ap()
    a_d = nc.dram_tensor("amat", [128, N2 * 128], F16, kind="ExternalInput").ap()
    # x: [128=(riI,m1), (h, m2, b'')]
    x_d = nc.dram_tensor("xin", [128, NH * N2 * BH], F16, kind="ExternalInput").ap()
    # output dump: per group g: [128=(b2l,k2,b0), (chl 16, riX 2, k1 64)]
    o_d = nc.dram_tensor("odump", [NH * 8 * 128, 1024], F16, kind="ExternalOutput").ap()

    x_v = x_d.rearrange("p (h m b) -> p h m b", h=NH, m=N2)
    o_v = o_d.rearrange("(g p) c -> g p c", g=NH * 8)

    with tile.TileContext(nc) as tc:
        with (
            tc.tile_pool(name="const", bufs=1) as cpool,
            tc.tile_pool(name="x", bufs=1) as xpool,
            tc.tile_pool(name="z", bufs=1) as zpool,
            tc.tile_pool(name="t", bufs=4) as tpool,
            tc.tile_pool(name="o", bufs=3) as opool,
        ):
            c_t = cpool.tile([128, 4, 128], F16, tag="cmat")
            a_t = cpool.tile([128, N2, 128], F16, tag="amat")
            x_t = xpool.tile([128, NH, N2, BH], F16, tag="x")
            ident = c_t[:, 0, :]

            # One FIFO DMA queue on the otherwise-idle SP engine: consts
            # first, then x half 0, x half 1, and (later) the output dumps.
            # FIFO order keeps output transfers from stealing bandwidth
            # from the input stream.
            nc.sync.dma_start(c_t[:], c_d.rearrange("p (s c) -> p s c", s=4))
            nc.sync.dma_start(a_t[:], a_d.rearrange("p (m c) -> p m c", m=N2))
            for h in range(NH):
                for q in range(N2 // 8):
                    nc.sync.dma_start(
                        x_t[:, h, 8 * q : 8 * q + 8, :], x_v[:, h, 8 * q : 8 * q + 8, :]
                    )

            # Z [128=(riO,k1), (h, b2, m2, b0)]
            z_t = zpool.tile([128, NH * BH * N2], F16, tag="z")
            # view for pair-interleaved stage-1 evac writes (m2, b2, b0 order)
            z_w = z_t[:].rearrange(
                "p (h b2 m b0) -> p h m b2 b0", h=NH, b2=BH // 2, m=N2
            )
            # view for transpose chunk reads (contiguous 128-col chunks)
            z_r = z_t[:].rearrange("p (h c w) -> p h c w", h=NH, c=NCH)

            sgre = c_t[:, 1, :]
            sgim = c_t[:, 2, :]
            sgimn = c_t[:, 3, :]

            with (
                tc.tile_pool(name="pst", bufs=3, space="PSUM") as pstpool,
                tc.tile_pool(name="ps2", bufs=5, space="PSUM") as ps2pool,
            ):
                # ---- warmup: junk matmuls on the identity during DMA fill ----
                for _ in range(NWARM):
                    psw = ps2pool.tile([128, 2, BH], F32, tag="pq")
                    for rep in range(4):
                        nc.tensor.matmul(
                            psw[:, rep // 2, 128 * (rep % 2) : 128 * (rep % 2 + 1)],
                            ident,
                            ident,
                            start=True,
                            stop=True,
                        )

                def emit_warm(n):
                    for _ in range(n):
                        pw = ps2pool.tile([128, 2, BH], F32, tag="pq", name="pw")
                        for rep in range(4):
                            nc.tensor.matmul(
                                pw[:, rep // 2, 128 * (rep % 2) : 128 * (rep % 2 + 1)],
                                ident,
                                ident,
                                start=True,
                                stop=True,
                            )

                def emit_s1(h, j0=0, j1=N2 // 2, weave_warm=False):
                    # stage 1 for half h: per m2-pair one PSUM tile + one evac
                    for j in range(j0, j1):
                        if weave_warm and j in (3, 6):
                            emit_warm(1)
                        ps = ps2pool.tile([128, 2, BH], F32, tag="pq")
                        nc.tensor.matmul(
                            ps[:, 0, :], a_t[:, 2 * j, :], x_t[:, h, 2 * j, :],
                            start=True, stop=True,
                        )
                        nc.tensor.matmul(
                            ps[:, 1, :], a_t[:, 2 * j + 1, :], x_t[:, h, 2 * j + 1, :],
                            start=True, stop=True,
                        )
                        # evac: in (m2l, b2, b0) contiguous; out pair-strided
                        src = ps[:].rearrange("p m (b2 b0) -> p m b2 b0", b0=2)
                        dst = z_w[:, h, 2 * j : 2 * j + 2, :, :]
                        if j % 2 == 0:
                            nc.vector.tensor_copy(dst, src)
                        else:
                            nc.scalar.copy(dst, src)

                def emit_tp(h, t):
                    # 8 transposes into one PSUM bank + one evac to T8.
                    # Transposes depend only on Z, so emitting them a group
                    # ahead keeps the PE fed while stage-2 waits on T8 evacs.
                    pt = pstpool.tile([128, 8, 128], F16, tag="pt")
                    for jj in range(8):
                        cl = t * 8 + jj
                        nc.tensor.transpose(
                            pt[:, jj, :], z_r[:, h, cl, :], ident
                        )
                    t8 = tpool.tile([128, 8 * 128], F16, tag="t8")
                    nc.vector.tensor_copy(t8[:], pt[:])
                    return t8

                def emit_s2(h, t, t8):
                    # 6 stage-2 matmuls (same-stationary batched to halve
                    # weight reloads), 2 plain evacs, one out-DMA
                    t8v = t8[:].rearrange("p (jj ri k) -> p jj ri k", jj=8, ri=2)
                    x_o = opool.tile([128, 8, 2, 64], F16, tag="xo")
                    pss = [
                        ps2pool.tile([128, 4, 2, 64], F32, tag="pq", name=f"pq_{h}_{t}_{s}")
                        for s in range(2)
                    ]
                    # X = S_re.T @ T, then accumulate the cross terms:
                    #   re-cols += (-S_im).T @ T[im-cols]
                    #   im-cols += S_im.T @ T[re-cols]
                    for s in range(2):
                        nc.tensor.matmul(
                            pss[s][:], sgre, t8[:, 512 * s : 512 * s + 512],
                            start=True, stop=False, skip_group_check=True,
                        )
                    for s in range(2):
                        nc.tensor.matmul(
                            pss[s][:, :, 0, :], sgimn,
                            t8v[:, 4 * s : 4 * s + 4, 1, :],
                            start=False, stop=False, skip_group_check=True,
                        )
                    for s in range(2):
                        nc.tensor.matmul(
                            pss[s][:, :, 1, :], sgim,
                            t8v[:, 4 * s : 4 * s + 4, 0, :],
                            start=False, stop=True, skip_group_check=True,
                        )
                    g = h * 8 + t
                    xof = x_o[:].rearrange("p c ri k -> p (c ri k)")
                    if (h, t) == (1, 7):
                        # final group: quarter evacs on DVE+ACT in parallel
                        # and 64KB dumps so the last-DMA-complete (the
                        # measured end of the kernel) lands ASAP
                        nc.vector.tensor_copy(x_o[:, 0:2, :, :], pss[0][:, 0:2])
                        nc.scalar.copy(x_o[:, 2:4, :, :], pss[0][:, 2:4])
                        nc.sync.dma_start(o_v[g][:, 0:256], xof[:, 0:256])
                        nc.gpsimd.dma_start(o_v[g][:, 256:512], xof[:, 256:512])
                        nc.vector.tensor_copy(x_o[:, 4:6, :, :], pss[1][:, 0:2])
                        nc.scalar.copy(x_o[:, 6:8, :, :], pss[1][:, 2:4])
                        nc.sync.dma_start(o_v[g][:, 512:768], xof[:, 512:768])
                        nc.gpsimd.dma_start(o_v[g][:, 768:1024], xof[:, 768:1024])
                    else:
                        if t % 2 == 0:
                            nc.vector.tensor_copy(x_o[:, 0:4, :, :], pss[0][:])
                        else:
                            nc.scalar.copy(x_o[:, 0:4, :, :], pss[0][:])
                        nc.sync.dma_start(o_v[g][:, 0:512], xof[:, 0:512])
                        nc.scalar.copy(x_o[:, 4:8, :, :], pss[1][:])
                        nc.gpsimd.dma_start(o_v[g][:, 512:1024], xof[:, 512:1024])

                # half 0 stage 1, then phase-2 h0 groups 0..3, then stage-1 h1
                # (x-h1 DMA has landed by then), then the rest
                emit_s1(0, weave_warm=True)
                sched = [(0, t) for t in range(8)] + [(1, t) for t in range(8)]
                from collections import deque

                t8q = deque()
                t8q.append(emit_tp(0, 0))
                t8q.append(emit_tp(0, 1))
                # all 16 h1 m2-pairs must be emitted before emit_tp(1, 0)
                # (at i=6) -- transposes read every m2 column of Z
                s1h1_plan = {0: (0, 2), 1: (2, 4), 2: (4, 7), 3: (7, 10), 4: (10, 13), 5: (13, 16)}
                for i, (h, t) in enumerate(sched):
                    if h == 0 and t in s1h1_plan:
                        emit_s1(1, *s1h1_plan[t])
                    if i + 2 < len(sched):
                        t8q.append(emit_tp(*sched[i + 2]))
                    emit_s2(h, t, t8q.popleft())

    nc.compile()
    return nc


def _consts():
    m1 = np.arange(N1, dtype=np.float64)
    k1 = np.arange(N1, dtype=np.float64)
    m2 = np.arange(N2, dtype=np.float64)
    k2 = np.arange(N2, dtype=np.float64)

    # amat[p=(riI*64+m1), m2*128 + (riO*64+k1)]
    # A_m2 = exp(-i*th), th = 2pi(32*m1+m2)k1/2048: Are=cos th, Aim=-sin th
    amat = np.empty((128, N2, 128), np.float64)
    for q in range(N2):
        th = 2.0 * np.pi * np.outer(32.0 * m1 + q, k1) / NFFT
        are = np.cos(th)
        aim = -np.sin(th)
        amat[0:64, q, 0:64] = are
        amat[64:128, q, 0:64] = -aim
        amat[0:64, q, 64:128] = aim
        amat[64:128, q, 64:128] = are

    # smat[w=(b2l*64+m2*2+b0), s*128 + (b2l'*64+k2*2+b0')]
    # W32 = exp(-i*phi), phi = 2pi*m2*k2/32: Wre=cos, Wim=-sin
    phi = 2.0 * np.pi * np.outer(m2, k2) / N2
    wre = np.cos(phi)
    wim = -np.sin(phi)
    w = np.arange(128)
    b2l_r = w // 64
    m2_r = (w % 64) // 2
    b0_r = w % 2
    cx = np.arange(128)
    b2l_c = cx // 64
    k2_c = (cx % 64) // 2
    b0_c = cx % 2
    mask = (b2l_r[:, None] == b2l_c[None, :]) & (b0_r[:, None] == b0_c[None, :])
    smat = np.zeros((128, 3, 128), np.float64)
    smat[:, 0, :] = wre[np.ix_(m2_r, k2_c)] * mask
    smat[:, 1, :] = wim[np.ix_(m2_r, k2_c)] * mask
    smat[:, 2, :] = -smat[:, 1, :]

    ident = np.eye(128, dtype=np.float64)
    cmat = np.concatenate([ident[:, :, None].transpose(0, 2, 1), smat], axis=1)
    return (
        np.ascontiguousarray(cmat.reshape(128, 4 * 128)).astype(np.float16),
        np.ascontiguousarray(amat.reshape(128, N2 * 128)).astype(np.float16),
    )


def run(signal_re, signal_im, trace=False, tmpdir=None):
    if "nc" not in _CACHE:
        _CACHE["nc"] = _build_nc()
        _CACHE["c"] = _consts()
    nc = _CACHE["nc"]
    cmat, amat = _CACHE["c"]

    sre = np.asarray(signal_re, dtype=np.float32).astype(np.float16)
    sim = np.asarray(signal_im, dtype=np.float32).astype(np.float16)

    in_maps = []
    for c in range(NCORES):
        bsl = slice(c * BPC, (c + 1) * BPC)
        # xin[riI*64+m1, h*8192 + m2*256 + b''] = x_ri[h*256+b'', 32*m1+m2]
        xr = sre[bsl].reshape(NH, BH, N1, N2)  # [h, b'', m1, m2]
        xi = sim[bsl].reshape(NH, BH, N1, N2)
        x = np.stack([xr, xi], axis=0)  # [ri, h, b'', m1, m2]
        x = x.transpose(0, 3, 1, 4, 2)  # [ri, m1, h, m2, b'']
        xin = np.ascontiguousarray(x.reshape(128, NH * N2 * BH))
        in_maps.append({"cmat": cmat, "amat": amat, "xin": xin})

    last_exc = None
    for attempt in range(3):
        try:
            br = run_bass_kernel_spmd(
                nc, in_maps, list(range(NCORES)), trace=trace, tmpdir=tmpdir
            )
            break
        except Exception as e:
            last_exc = e
            import time

            time.sleep(2.0)
    else:
        raise last_exc

    out_re = np.empty((BATCH, NFFT), np.float32)
    out_im = np.empty((BATCH, NFFT), np.float32)
    for c in range(NCORES):
        bsl = slice(c * BPC, (c + 1) * BPC)
        # odump[g*128 + (b2l*64+k2*2+b0), chl*128 + riX*64 + k1]
        # b = 4*(g*8+chl) + 2*b2l + b0 ; k = k1 + 64*k2
        d = br.results[c]["odump"].reshape(16, 2, 32, 2, 8, 2, 64)
        # dims: [g, b2l, k2, b0, chl, riX, k1]
        arr = d.transpose(5, 0, 4, 1, 3, 2, 6).reshape(2, BPC, NFFT)
        out_re[bsl, :] = arr[0].astype(np.float32)
        out_im[bsl, :] = arr[1].astype(np.float32)
    return (out_re, out_im), br


def kernel(signal_re, signal_im):
    return run(signal_re, signal_im)[0]
